# revision 1
# baseline (speedup 1.0000x reference)
"""AttnBlock for Trainium2, 8 NeuronCores — fp8e4 DoubleRow rewrite (v2).

Sharding: core i = (batch i//2, query-half i%2). Full K/V per core, no
collectives. One program for all cores: odd cores get the image columns
rolled by 2048 (attention is permutation-equivariant over key positions;
GroupNorm stats are order-invariant), so every core computes queries 0..2047
of its (possibly rolled) image.

Math (exact rearrangement of the reference):
  GroupNorm h = scale*x + shift; scale is folded into the fp8 conv WEIGHTS
  on device (w' = fp8(w8 * scale_cin)), so the x->fp8 casts need no scale
  and run chunk-by-chunk behind the input DMA, before stats complete.
  shift contributions: k-conv -> constant along m, dropped with bk (softmax
  invariant); v-conv -> v0 = Wv shift + bv contributes Wo v0 to every output
  (softmax rows sum to 1) -> fbias = Wo v0 + bo, pre-added into x in place;
  q-conv -> q0 = Wq shift + bq added in the q epilogue. 1/sqrt(C) is applied
  in the k epilogue.

All heavy matmuls are fp8e4 DoubleRow (0.5 cyc/row, K=256/instruction).
P=exp(scores) is cast to fp8 by Act directly from PSUM, two half-tile ops
per m-chunk so the next chunk's score matmuls only wait on the half they
overwrite (subtile deps). PV and the softmax denominator accumulate in PSUM
across all of m; the denominator is a DoubleRow ones-matmul whose stationary
[128,2,128] of ones broadcasts the result across all partitions, making the
final divide a plain tensor_tensor. Proj + residual are deferred one n-chunk
and interleaved into the next chunk's matmul stream.

PSUM budget (8 banks): scores [128,4,256] (2), PV [128,2x2,512-padded] (4,
one open accumulation group per bank), den [128,256] (1), proj [128,256] (1).
"""

import math
import os
import sys

sys.path.insert(0, "/opt/trn_rl_repo")

import numpy as np
import ml_dtypes

import concourse.bacc as bacc
import concourse.bass as bass
import concourse.mybir as mybir
import concourse.tile as tile
from concourse.bass_utils import run_bass_kernel_spmd

F32 = mybir.dt.float32
F32R = mybir.dt.float32r
FP8 = mybir.dt.float8e4
DR = mybir.MatmulPerfMode.DoubleRow
MULT = mybir.AluOpType.mult
ADD = mybir.AluOpType.add
SUB = mybir.AluOpType.subtract
EXP = mybir.ActivationFunctionType.Exp
IDENT = mybir.ActivationFunctionType.Identity
COPY = mybir.ActivationFunctionType.Copy
SQRT = mybir.ActivationFunctionType.Sqrt

B, C, H, W = 4, 512, 64, 64
HW = H * W
G = 32
GS = C // G
NQ = HW // 2
EPS = 1e-5
N_CORES = 8
CT = C // 128
MC = HW // 512          # 8 m-chunks
NC = NQ // 256          # 8 n-chunks
INV_SQRT_C = 1.0 / math.sqrt(C)

LAST_RESULTS = None


def _build():
    nc = bacc.Bacc("TRN2", target_bir_lowering=False, debug=False)

    x_d = nc.dram_tensor("x_img", [C, HW], F32R, kind="ExternalInput").ap()
    w_st = {n: nc.dram_tensor(n, [128, 2, 4, 2, 128], FP8, kind="ExternalInput").ap()
            for n in ("wq8", "wk8", "wo8")}
    wv8m_d = nc.dram_tensor("wv8m", [128, 2, 2, 512], FP8, kind="ExternalInput").ap()
    mg_d = nc.dram_tensor("Mg", [C, G], F32, kind="ExternalInput").ap()
    m2_d = nc.dram_tensor("M2", [G, C], F32, kind="ExternalInput").ap()
    # gamma, beta, bq, bv, bo packed as one [5, C] tensor -> [128, 5, CT] cols
    v5_d = nc.dram_tensor("vec5", [5, C], F32, kind="ExternalInput").ap()
    out_d = nc.dram_tensor("out", [C, NQ], F32, kind="ExternalOutput").ap()

    rx = x_d.rearrange("(t p) m -> p t m", p=128)
    rout = out_d.rearrange("(t p) n -> p t n", p=128)

    with tile.TileContext(nc) as tc:
        with (
            tc.tile_pool(name="singles", bufs=1) as singles,
            tc.tile_pool(name="statp", bufs=2) as statp,
            tc.tile_pool(name="p8p", bufs=3) as p8p,
            tc.tile_pool(name="ao8p", bufs=2) as ao8p,
            tc.tile_pool(name="otp", bufs=2) as otp,
        ):
            # ---------------- constants / small loads ----------------
            v5 = singles.tile([128, 5, CT], F32, tag="v5")
            nc.sync.dma_start(out=v5, in_=v5_d.rearrange("v (t p) -> p v t", p=128))
            cols = {n: v5[:, i, :]
                    for i, n in enumerate(("gamma", "beta", "bq", "bv", "bo"))}
            eps_t = singles.tile([G, 1], F32, tag="eps")
            nc.vector.memset(eps_t, EPS)
            ones8 = singles.tile([128, 2, 128], FP8, tag="ones8")
            nc.vector.memset(ones8, 1.0)
            Mg = singles.tile([128, CT, G], F32, tag="Mg")
            nc.sync.dma_start(out=Mg, in_=mg_d.rearrange("(t p) g -> p t g", p=128))
            M2 = singles.tile([G, CT, 128], F32, tag="M2")
            nc.sync.dma_start(out=M2, in_=m2_d.rearrange("g (t p) -> g t p", p=128))

            w8 = {}
            for n in ("wq8", "wk8", "wo8"):
                t = singles.tile([128, 2, 4, 2, 128], FP8, tag=n, name=n)
                nc.scalar.dma_start(out=t, in_=w_st[n])
                w8[n] = t
            wv8 = singles.tile([128, 2, 2, 512], FP8, tag="wv8m")
            nc.scalar.dma_start(out=wv8, in_=wv8m_d)

            # ------ pass 1: x DMA; per chunk: bn_stats (DVE) + x8 casts (Pool)
            x_t = singles.tile([128, CT, HW], F32R, tag="x_t")
            stats_all = singles.tile([128, CT, MC, 6], F32, tag="stats_all")
            # x8a: [128, j, ctp, m] conv moving; x8v: [128, j, mt, ctp, 128]
            # v-conv stationary (pair blocks contiguous)
            x8a = singles.tile([128, 2, 2, HW], FP8, tag="x8a")
            x8v = singles.tile([128, 2, HW // 128, 2, 128], FP8, tag="x8v")
            for mc in range(MC):
                ms = slice(mc * 512, (mc + 1) * 512)
                nc.sync.dma_start(out=x_t[:, :, ms], in_=rx[:, :, ms])
                for t in range(CT):
                    nc.vector.bn_stats(out=stats_all[:, t, mc, :], in_=x_t[:, t, ms])
                if mc % 2 == 1:
                    m2s = slice((mc - 1) * 512, (mc + 1) * 512)
                    for t in range(CT):
                        nc.scalar.activation(out=x8a[:, t // 2, t % 2, m2s],
                                             in_=x_t[:, t, m2s], func=COPY)
                        nc.gpsimd.tensor_copy(
                            out=x8v[:, t // 2, 4 * mc - 4:4 * mc + 4, t % 2, :],
                            in_=x_t[:, t, m2s].rearrange("p (mt m) -> p mt m", m=128))
            mv = statp.tile([128, CT, 2], F32, tag="mv")
            for t in range(CT):
                nc.vector.bn_aggr(out=mv[:, t, :], in_=stats_all[:, t, :, :])
            s_cat = statp.tile([128, CT, 2], F32, tag="s_cat")
            nc.vector.tensor_copy(out=s_cat[:, :, 0:1], in_=mv[:, :, 0:1])
            nc.vector.tensor_tensor(s_cat[:, :, 1:2], mv[:, :, 0:1], mv[:, :, 0:1], MULT)
            nc.vector.tensor_tensor(s_cat[:, :, 1:2], s_cat[:, :, 1:2], mv[:, :, 1:2], ADD)

            k8 = singles.tile([128, 2, HW // 128, 2, 128], FP8, tag="k8")
            vT8 = singles.tile([128, HW // 256, CT, 2, 128], FP8, tag="vT8")
            q8 = singles.tile([128, 2, NC, 2, 256], FP8, tag="q8")

            with tc.tile_pool(name="ps_a", bufs=4, space="PSUM") as ps_a:
                gsum_ps = ps_a.tile([128, 512], F32, tag="cps", name="gsum_ps")
                for ct in range(CT):
                    nc.tensor.matmul(gsum_ps[0:G, 0:2], Mg[:, ct, :], s_cat[:, ct, :],
                                     start=(ct == 0), stop=(ct == CT - 1))
                gmean = statp.tile([G, 1], F32, tag="gmean")
                ge2 = statp.tile([G, 1], F32, tag="ge2")
                nc.vector.tensor_scalar_mul(gmean, gsum_ps[0:G, 0:1], 1.0 / GS)
                nc.vector.tensor_scalar_mul(ge2, gsum_ps[0:G, 1:2], 1.0 / GS)
                gvar = statp.tile([G, 1], F32, tag="gvar")
                nc.vector.tensor_tensor(gvar, gmean, gmean, MULT)
                nc.vector.tensor_tensor(gvar, ge2, gvar, SUB)
                grstd = statp.tile([G, 2], F32, tag="grstd")
                nc.scalar.activation(out=gvar, in_=gvar, func=SQRT, bias=eps_t, scale=1.0)
                nc.vector.reciprocal(grstd[:, 0:1], gvar)
                nc.vector.tensor_copy(out=grstd[:, 1:2], in_=gmean)
                rm_pc = statp.tile([128, CT, 2], F32, tag="rm_pc")
                for ct in range(CT):
                    rm_ps = ps_a.tile([128, 512], F32, tag="cps", name="rm_ps")
                    nc.tensor.matmul(rm_ps[:, 0:2], M2[:, ct, :], grstd,
                                     start=True, stop=True)
                    nc.vector.tensor_copy(out=rm_pc[:, ct, :], in_=rm_ps[:, 0:2])
                scale_pc = singles.tile([128, CT], F32, tag="scale_pc")
                shift_pc = singles.tile([128, CT], F32, tag="shift_pc")
                nc.vector.tensor_tensor(scale_pc, cols["gamma"], rm_pc[:, :, 0], MULT)
                nc.vector.tensor_tensor(shift_pc, scale_pc, rm_pc[:, :, 1], MULT)
                nc.vector.tensor_tensor(shift_pc, cols["beta"], shift_pc, SUB)

                # ---- matvecs on UNscaled weights: q0, v0, fbias ----
                shift8 = singles.tile([128, CT, 1], FP8, tag="shift8")
                nc.vector.tensor_scalar_mul(shift8[:, :, 0], shift_pc, 1.0)
                q0col = singles.tile([128, CT], F32, tag="q0col")
                v0col = singles.tile([128, CT], F32, tag="v0col")
                v08 = singles.tile([128, CT, 1], FP8, tag="v08")
                fbias = singles.tile([128, CT], F32, tag="fbias")
                mv_ps = ps_a.tile([128, 512], F32, tag="cps", name="mv_ps")
                for dt in range(CT):
                    for ct in range(CT):
                        nc.tensor.matmul(mv_ps[:, dt:dt + 1],
                                         w8["wq8"][:, ct // 2, dt, ct % 2, :],
                                         shift8[:, ct, :],
                                         start=(ct == 0), stop=(ct == CT - 1))
                nc.vector.tensor_tensor(q0col, mv_ps[:, 0:CT], cols["bq"], ADD)
                mv_ps2 = ps_a.tile([128, 512], F32, tag="cps", name="mv_ps2")
                for dt in range(CT):
                    for ct in range(CT):
                        nc.tensor.matmul(mv_ps2[:, dt:dt + 1],
                                         wv8[:, ct // 2, ct % 2,
                                             dt * 128:(dt + 1) * 128],
                                         shift8[:, ct, :],
                                         start=(ct == 0), stop=(ct == CT - 1))
                nc.vector.tensor_tensor(v0col, mv_ps2[:, 0:CT], cols["bv"], ADD)
                nc.vector.tensor_scalar_mul(v08[:, :, 0], v0col, 1.0)
                mv_ps3 = ps_a.tile([128, 512], F32, tag="cps", name="mv_ps3")
                for dt in range(CT):
                    for ct in range(CT):
                        nc.tensor.matmul(mv_ps3[:, dt:dt + 1],
                                         w8["wo8"][:, ct // 2, dt, ct % 2, :],
                                         v08[:, ct, :],
                                         start=(ct == 0), stop=(ct == CT - 1))
                nc.vector.tensor_tensor(fbias, mv_ps3[:, 0:CT], cols["bo"], ADD)

                # ---- scaled copies of q/k/v weights (Pool, SBUF fp8);
                #      originals stay for the matvecs, so no WAR chain ----
                wk8s = singles.tile([128, 2, 4, 2, 128], FP8, tag="wk8s")
                wv8s = singles.tile([128, 2, 2, 512], FP8, tag="wv8s")
                wq8s = singles.tile([128, 2, 4, 2, 128], FP8, tag="wq8s")
                for ct in range(CT):
                    j, p = ct // 2, ct % 2
                    nc.gpsimd.tensor_scalar(
                        wk8s[:, j, :, p, :], w8["wk8"][:, j, :, p, :],
                        scale_pc[:, ct:ct + 1], None, MULT)
                for ct in range(CT):
                    j, p = ct // 2, ct % 2
                    nc.gpsimd.tensor_scalar(
                        wv8s[:, j, p, :], wv8[:, j, p, :],
                        scale_pc[:, ct:ct + 1], None, MULT)
                for ct in range(CT):
                    j, p = ct // 2, ct % 2
                    nc.gpsimd.tensor_scalar(
                        wq8s[:, j, :, p, :], w8["wq8"][:, j, :, p, :],
                        scale_pc[:, ct:ct + 1], None, MULT)

                # ---------------- convs ----------------
                for mc in range(MC):
                    ms = slice(mc * 512, (mc + 1) * 512)
                    for dt in range(CT):
                        cps = ps_a.tile([128, 512], F32, tag="cps", name="kc")
                        for j in range(2):
                            nc.tensor.matmul(cps, wk8s[:, j, dt],
                                             x8a[:, j, :, ms],
                                             start=(j == 0), stop=(j == 1),
                                             perf_mode=DR)
                        if dt % 2 == 0:
                            nc.vector.tensor_scalar(
                                k8[:, dt // 2, 4 * mc:4 * mc + 4, dt % 2, :],
                                cps.rearrange("p (mt m) -> p mt m", m=128),
                                INV_SQRT_C, None, MULT)
                        else:
                            nc.scalar.activation(
                                out=k8[:, dt // 2, 4 * mc:4 * mc + 4, dt % 2, :],
                                in_=cps.rearrange("p (mt m) -> p mt m", m=128),
                                func=COPY, scale=INV_SQRT_C)
                    for msub in range(4):
                        g = 4 * mc + msub
                        cps = ps_a.tile([128, 512], F32, tag="cps", name="vc")
                        for j in range(2):
                            nc.tensor.matmul(cps, x8v[:, j, g], wv8s[:, j],
                                             start=(j == 0), stop=(j == 1),
                                             perf_mode=DR)
                        if msub % 2 == 0:
                            nc.scalar.activation(
                                out=vT8[:, g // 2, :, g % 2, :],
                                in_=cps.rearrange("p (ct m) -> p ct m", m=128),
                                func=COPY)
                        else:
                            nc.vector.tensor_copy(
                                out=vT8[:, g // 2, :, g % 2, :],
                                in_=cps.rearrange("p (ct m) -> p ct m", m=128))
                for t in range(CT):
                    ns = slice(t * 512, (t + 1) * 512)
                    for dt in range(CT):
                        cps = ps_a.tile([128, 512], F32, tag="cps", name="qc")
                        for j in range(2):
                            nc.tensor.matmul(cps, wq8s[:, j, dt],
                                             x8a[:, j, :, ns],
                                             start=(j == 0), stop=(j == 1),
                                             perf_mode=DR)
                        if dt % 2 == 0:
                            nc.scalar.activation(
                                out=q8[:, dt // 2, 2 * t:2 * t + 2, dt % 2, :],
                                in_=cps.rearrange("p (two n) -> p two n", n=256),
                                func=IDENT, bias=q0col[:, dt:dt + 1], scale=1.0)
                        else:
                            nc.vector.tensor_scalar(
                                q8[:, dt // 2, 2 * t:2 * t + 2, dt % 2, :],
                                cps.rearrange("p (two n) -> p two n", n=256),
                                q0col[:, dt:dt + 1], None, ADD)

                # residual prep: x += fbias on query columns (DVE, off the
                # conv critical path; consumed only by main-loop tails)
                for ct in range(CT):
                    nc.gpsimd.tensor_scalar(
                        x_t[:, ct, 0:NQ], x_t[:, ct, 0:NQ],
                        fbias[:, ct:ct + 1], None, ADD)

            # ---------------- main attention loop ----------------
            with (
                tc.tile_pool(name="ps_s", bufs=3, space="PSUM") as ps_s,
                tc.tile_pool(name="ps_pv", bufs=1, space="PSUM") as ps_pv,
            ):
                pending = None  # (nci, ao8)

                def tail_step(dts, ot):
                    pnci, pao8 = pending
                    for dt in dts:
                        pr_t = ps_s.tile([128, 2, 256], F32, tag="s", name="pr")
                        pr_ps = pr_t[:, 0, :]
                        for j in range(2):
                            nc.tensor.matmul(pr_ps, w8["wo8"][:, j, dt],
                                             pao8[:, 2 * j:2 * j + 2, :],
                                             start=(j == 0), stop=(j == 1),
                                             perf_mode=DR)
                        nc.vector.tensor_tensor(
                            ot[:, dt, :], pr_ps,
                            x_t[:, dt, pnci * 256:(pnci + 1) * 256], ADD)

                def tail_flush(ot):
                    pnci = pending[0]
                    nc.sync.dma_start(
                        out=rout[:, :, pnci * 256:(pnci + 1) * 256], in_=ot)

                for nci in range(NC):
                    p_tiles = []
                    # one open accumulation group per bank: [128, ct-pair*2, 512]
                    pv_a = ps_pv.tile([128, 2, 512], F32, tag="pva", name="pva")
                    pv_b = ps_pv.tile([128, 2, 512], F32, tag="pvb", name="pvb")
                    pv = [pv_a[:, 0, 0:256], pv_a[:, 1, 0:256],
                          pv_b[:, 0, 0:256], pv_b[:, 1, 0:256]]
                    den_ps = ps_pv.tile([128, 256], F32, tag="den", name="den")
                    ot = otp.tile([128, CT, 256], F32, tag="ot", name="ot") \
                        if pending is not None else None
                    for mc in range(MC):
                        p8 = p8p.tile([128, 4, 256], FP8, tag="p8", name="p8")
                        p_tiles.append(p8)
                        for half in range(2):
                            hs = slice(2 * half, 2 * half + 2)
                            s_ps = ps_s.tile([128, 2, 256], F32, tag="s", name="s_ps")
                            for t in (0, 1):
                                msub = 2 * half + t
                                for j in range(2):
                                    nc.tensor.matmul(
                                        s_ps[:, t, :], k8[:, j, 4 * mc + msub],
                                        q8[:, j, nci],
                                        start=(j == 0), stop=(j == 1), perf_mode=DR)
                            nc.scalar.activation(out=p8[:, hs, :], in_=s_ps,
                                                 func=EXP)
                        if mc > 0:
                            pp = p_tiles[mc - 1]
                            pm = mc - 1
                            for u in range(2):
                                for ct in range(CT):
                                    nc.tensor.matmul(
                                        pv[ct], vT8[:, 2 * pm + u, ct],
                                        pp[:, 2 * u:2 * u + 2, :],
                                        start=(pm == 0 and u == 0), stop=False,
                                        perf_mode=DR)
                                nc.tensor.matmul(
                                    den_ps, ones8, pp[:, 2 * u:2 * u + 2, :],
                                    start=(pm == 0 and u == 0), stop=False,
                                    perf_mode=DR)
                        if pending is not None and mc % 2 == 1:
                            tail_step([mc // 2], ot)
                            if mc == MC - 1:
                                tail_flush(ot)
                                pending = None
                    pp = p_tiles[MC - 1]
                    pm = MC - 1
                    for u in range(2):
                        for ct in range(CT):
                            nc.tensor.matmul(
                                pv[ct], vT8[:, 2 * pm + u, ct],
                                pp[:, 2 * u:2 * u + 2, :],
                                start=False, stop=(u == 1), perf_mode=DR)
                        nc.tensor.matmul(den_ps, ones8, pp[:, 2 * u:2 * u + 2, :],
                                         start=False, stop=(u == 1), perf_mode=DR)
                    rec = statp.tile([128, 256], F32, tag="rec", name="rec")
                    nc.vector.reciprocal(rec, den_ps)
                    ao8 = ao8p.tile([128, CT, 256], FP8, tag="ao8", name="ao8")
                    for ct in range(CT):
                        nc.vector.tensor_tensor(ao8[:, ct, :], pv[ct], rec, MULT)
                    pending = (nci, ao8)
                ot = otp.tile([128, CT, 256], F32, tag="ot", name="ot_f")
                tail_step(list(range(CT)), ot)
                tail_flush(ot)
    nc.finalize()
    return nc


_NC_CACHE = {}


def _get_nc():
    if "nc" not in _NC_CACHE:
        _NC_CACHE["nc"] = _build()
    return _NC_CACHE["nc"]


def _prep_stationary(w):
    # w: [cout, cin] conv weight -> stationary DR layout [p, j, dt, ctp, m]
    wT = np.ascontiguousarray(w.T)                      # [cin, cout]
    arr = wT.reshape(2, 2, 128, 4, 128)                  # [j, ctp, p, dt, m]
    arr = np.transpose(arr, (2, 0, 3, 1, 4))             # [p, j, dt, ctp, m]
    return np.ascontiguousarray(arr).astype(ml_dtypes.float8_e4m3)


def _prep_moving(w):
    # w: [cout, cin] -> moving DR layout [p, j, ctp, cout]
    wT = np.ascontiguousarray(w.T)                      # [cin, cout]
    arr = wT.reshape(2, 2, 128, 512)                     # [j, ctp, p, cout]
    arr = np.transpose(arr, (2, 0, 1, 3))                # [p, j, ctp, cout]
    return np.ascontiguousarray(arr).astype(ml_dtypes.float8_e4m3)


def kernel(**inputs):
    x = np.ascontiguousarray(np.asarray(inputs["x"], dtype=np.float32))
    gamma = np.asarray(inputs["gamma"], np.float32)
    beta = np.asarray(inputs["beta"], np.float32)
    w = {n: np.asarray(inputs[n], np.float32) for n in ("wq", "wk", "wv", "wo")}
    b = {n: np.asarray(inputs[n], np.float32) for n in ("bq", "bk", "bv", "bo")}

    mg_np = np.zeros((C, G), np.float32)
    mg_np[np.arange(C), np.arange(C) // GS] = 1.0
    common = {
        "Mg": mg_np,
        "M2": np.ascontiguousarray(mg_np.T),
        "wq8": _prep_stationary(w["wq"]),
        "wk8": _prep_stationary(w["wk"]),
        "wo8": _prep_stationary(w["wo"]),
        "wv8m": _prep_moving(w["wv"]),
        "vec5": np.ascontiguousarray(
            np.stack([gamma, beta, b["bq"], b["bv"], b["bo"]])),
    }
    in_maps = []
    for core in range(N_CORES):
        bi, ch = divmod(core, 2)
        xi = x[bi].reshape(C, HW)
        if ch:
            xi = np.roll(xi, -NQ, axis=1)
        m = dict(common)
        m["x_img"] = np.ascontiguousarray(xi)
        in_maps.append(m)

    want_trace = bool(int(os.environ.get("KTRACE", "0")))
    if not want_trace:
        os.environ["BASS_NEVER_TRACE"] = "1"
    global LAST_RESULTS
    LAST_RESULTS = run_bass_kernel_spmd(
        _get_nc(), in_maps, core_ids=list(range(N_CORES)), trace=want_trace)
    full = np.empty((B, C, HW), np.float32)
    for core in range(N_CORES):
        bi, ch = divmod(core, 2)
        full[bi][:, ch * NQ:(ch + 1) * NQ] = LAST_RESULTS.results[core]["out"]
    return full.reshape(B, C, H, W)

